# revision 1
# baseline (speedup 1.0000x reference)
"""ConvCNP encoder kernel for 8x TRN2 NeuronCores.

Math: the reference computes, for a 128x128 uniform grid g=(xs[i], ys[j]) and
n=8192 data points X (2-D) with values psi(Y) = [1, Y0, Y1]:

    Gram[g, x] = exp(-0.5*||g - X[x]||^2)
    fm = Gram @ psi                  # (G, 3); column 0 == row-sum (denominator)
    out[c, j, i] = fm[(i, j), c], with c=1,2 normalized by column 0.

The squared distance is separable over the grid axes:

    Gram[(i,j), x] = A[i, x] * B[j, x]
      A[i, x] = exp(-0.5*(xs[i] - X0[x])^2)     B[j, x] = exp(-0.5*(ys[j] - X1[x])^2)

so, with Bc = B * psi_c (row-wise):  fm[(i,j), c] = sum_x Bc[j, x] * A[i, x].

Sharding: grid y-axis (j) across the 8 cores - 16 j-rows per core; X, Y
replicated. No cross-core communication. Per core:

    acc[(c,j), i] = sum over 64 x-chunks of  BfT_k^T @ AT_k      (PE, PSUM accum)
      AT_k  = exp(-0.5*(xs[i] - X0[x])^2)  in SBUF layout [x_part=128, i=128]
      BfT_k = [B | B*Y0 | B*Y1]            in SBUF layout [x_part=128, 48]

AT is produced by a fused custom DVE op  sq(Src0 - Src1)  over broadcast APs
(one 1x pass) followed by one big ACT Exp (scale=-0.5) per stripe. The Gram
factors are stored as fp16 (the fp32 argument keeps exp accuracy; fp16 values
feed the PE at 1 cycle/column instead of fp32's two 4-cycle passes).
"""

import numpy as np
from contextlib import ExitStack

N_AXIS = 128          # grid points per axis
NPTS = 8192           # data points
NCORES = 8
JS = N_AXIS // NCORES  # 16 grid-y rows per core
NCHUNK = NPTS // 128   # 64 contraction chunks of 128
STRIPE = 16            # chunks per elementwise stripe
NSTRIPES = NCHUNK // STRIPE
GRID_LO, GRID_HI = -2.0, 2.0

_CACHE = {}


def _register_sqdiff():
    """Register a fused (a-b)^2 custom DVE op (idempotent)."""
    from concourse import dve_ops
    from concourse.dve_spec import Spec, Src0, Src1, sq, lower
    from concourse.dve_uop import DveOpSpec

    name = "TENSOR_SQDIFF_X"
    for op in dve_ops.OPS:
        if op.name == name:
            return op
    spec = Spec(
        body=sq(Src0 - Src1),
        reference=lambda in0, in1, s0, s1, imm2: (in0.astype(np.float32) - in1) ** 2,
    )
    opcode = max(dve_ops._SUB_OPCODE_FOR_NAME.values()) + 1
    assert opcode < 0x20
    dve_ops._SUB_OPCODE_FOR_NAME[name] = opcode
    shas = {}
    for ver in ("v3", "v4"):
        s = DveOpSpec(name=name, opcode=opcode, uops=lower(spec, ver=ver), rd1_en=True)
        shas[ver] = s.sha(ver)
    op = dve_ops.DveOp(name, spec, subdim=False, uops_sha=shas)
    dve_ops.OPS.append(op)
    dve_ops.CUSTOM_DVE_SPECS[name] = spec
    return op


def _patch_walrus_flags():
    """Cap the compiler's semaphore file so the NEFF epilogue restores ~176
    semaphores instead of all 254 (the restore is ~40ns/sem/engine of pure
    tail latency). Idempotent."""
    import concourse.bass_utils as bu

    if getattr(bu.run_command, "_sem_cap_patched", False):
        return
    orig = bu.run_command

    def run_command_capped(argv, **kwargs):
        if argv and "walrus_driver" in str(argv[0]) and any(
                str(a).startswith("--neff-output-filename") for a in argv):
            argv = list(argv) + ["--max-sem-num=176"]
        return orig(argv, **kwargs)

    run_command_capped._sem_cap_patched = True
    bu.run_command = run_command_capped


def _build_program():
    import concourse.bacc as bacc
    import concourse.mybir as mybir
    import concourse.tile as tile

    _patch_walrus_flags()
    sqdiff = _register_sqdiff()

    f32 = mybir.dt.float32
    f16 = mybir.dt.float16
    nc = bacc.Bacc("TRN2", target_bir_lowering=False, debug=False, num_devices=NCORES,
                   enable_partition_id=False, monotonic_sem_count=0)

    # Packed inputs (fewer, earlier DMAs):
    #   bc [128, 80]  f32: x1t(0:64) | ysb(64:80)     -> unblocks the B chain
    #   ac [128, 192] f32: xsb(0:128) | x0t(128:192)  -> A stripes
    #   yc [128, 128] f16: y0t(0:64)  | y1t(64:128)   -> B*psi muls
    bc = nc.dram_tensor("bc", [128, 80], f32, kind="ExternalInput")
    ac = nc.dram_tensor("ac", [128, 192], f32, kind="ExternalInput")
    yc = nc.dram_tensor("yc", [128, 128], f16, kind="ExternalInput")
    out = nc.dram_tensor("out", [128, 3 * JS], f32, kind="ExternalOutput")

    with tile.TileContext(nc) as tc, ExitStack() as ctx:
        singles = ctx.enter_context(tc.tile_pool(name="singles", bufs=1))
        argp = ctx.enter_context(tc.tile_pool(name="argp", bufs=3))
        atp = ctx.enter_context(tc.tile_pool(name="atp", bufs=3))
        psum = ctx.enter_context(tc.tile_pool(name="psum", bufs=1, space="PSUM"))

        s_bc = singles.tile([128, 80], f32, tag="bc")
        nc.sync.dma_start(s_bc[:, :], bc[:, :])
        s_ac = singles.tile([128, 192], f32, tag="ac")
        nc.sync.dma_start(s_ac[:, :], ac[:, :])
        s_yc = singles.tile([128, 128], f16, tag="yc")
        nc.gpsimd.dma_start(s_yc[:, :], yc[:, :])

        x1t = s_bc[:, 0:64]
        ysb = s_bc[:, 64:80]
        xsb = s_ac[:, 0:128]
        x0t = s_ac[:, 128:192]

        # ---- B side: BfT[x_p, k, 0:48] = [B | B*Y0 | B*Y1], all 64 chunks ----
        s_bsq = singles.tile([128, NCHUNK, JS], f32, tag="bsq")
        s_bf = singles.tile([128, NCHUNK, 3 * JS], f16, tag="bf")

        nc.vector._custom_dve(
            sqdiff,
            out=s_bsq[:, :, :],
            in0=ysb.unsqueeze(1).broadcast_to([128, NCHUNK, JS]),
            in1=x1t.unsqueeze(2).broadcast_to([128, NCHUNK, JS]),
        )
        nc.scalar.activation(
            s_bf[:, :, 0:JS], s_bsq[:, :, :],
            mybir.ActivationFunctionType.Exp, scale=-0.5,
        )

        # ---- A side + matmul, striped; B*psi muls slotted after stripe 0 ----
        # acc[i, (c, j)]: lhsT = AT chunk (128 fp16 weight cols), rhs = BfT
        # chunk [128, 48]. (c, j) on the free axis makes the normalization a
        # plain broadcast-AP multiply.
        acc = psum.tile([128, 3 * JS], f32, tag="acc")
        stripe_sizes = [16, 16, 16, 8, 8]
        assert sum(stripe_sizes) == NCHUNK
        k0 = 0
        for s, width in enumerate(stripe_sizes):
            arg = argp.tile([128, STRIPE, 128], f32, tag="arg", name="arg")[:, 0:width, :]
            nc.vector._custom_dve(
                sqdiff,
                out=arg[:, :, :],
                in0=xsb.unsqueeze(1).broadcast_to([128, width, 128]),
                in1=x0t[:, k0:k0 + width].unsqueeze(2).broadcast_to(
                    [128, width, 128]),
            )
            at = atp.tile([128, STRIPE, 128], f16, tag="at", name="at")[:, 0:width, :]
            nc.scalar.activation(
                at[:, :, :], arg[:, :, :],
                mybir.ActivationFunctionType.Exp, scale=-0.5,
            )
            if s == 0:
                # B*Y0, B*Y1 (DVE, 1x with the broadcast psi operand) — after
                # stripe 0 so the DVE isn't stalled on the B exp, before the
                # first matmul needs the full 48-column BfT.
                for c in range(2):
                    nc.vector.tensor_tensor(
                        s_bf[:, :, (c + 1) * JS:(c + 2) * JS], s_bf[:, :, 0:JS],
                        s_yc[:, c * NCHUNK:(c + 1) * NCHUNK].unsqueeze(2)
                            .broadcast_to([128, NCHUNK, JS]),
                        mybir.AluOpType.mult,
                    )
            for k in range(width):
                nc.tensor.matmul(
                    acc[:, :],
                    at[:, k, :],         # stationary lhsT: [128, 128] fp16
                    s_bf[:, k0 + k, :],  # moving rhs: [128, 48] fp16
                    start=(k0 + k == 0),
                    stop=(k0 + k == NCHUNK - 1),
                )
            k0 += width

        # ---- epilogue: normalize columns 1,2 by column 0 (the row-sum) ----
        s_rec = singles.tile([128, JS], f32, tag="rec")
        nc.vector.reciprocal_approx_fast(s_rec[:, :], acc[:, 0:JS])
        s_out = singles.tile([128, 3 * JS], f32, tag="outt")
        nc.vector.tensor_copy(s_out[:, 0:JS], acc[:, 0:JS])
        nc.vector.tensor_tensor(
            s_out[:, JS:3 * JS].rearrange("p (c j) -> p c j", c=2),
            acc[:, JS:3 * JS].rearrange("p (c j) -> p c j", c=2),
            s_rec[:, :].unsqueeze(1).broadcast_to([128, 2, JS]),
            mybir.AluOpType.mult,
        )
        nc.sync.dma_start(out[:, :], s_out[:, :])

    nc.finalize()
    return nc


def _get_program():
    if "nc" not in _CACHE:
        _CACHE["nc"] = _build_program()
    return _CACHE["nc"]


def _host_inputs(X, Y):
    """Build the per-core input maps (layout prep only)."""
    X = np.ascontiguousarray(np.asarray(X, dtype=np.float32))
    Y = np.ascontiguousarray(np.asarray(Y, dtype=np.float32))
    xs = np.linspace(GRID_LO, GRID_HI, N_AXIS, dtype=np.float32)
    ys = np.linspace(GRID_LO, GRID_HI, N_AXIS, dtype=np.float32)

    ac = np.empty((128, 192), np.float32)
    ac[:, 0:128] = xs[None, :]
    ac[:, 128:192] = X[:, 0].reshape(NCHUNK, 128).T
    yc = np.empty((128, 128), np.float16)
    yc[:, 0:64] = Y[:, 0].reshape(NCHUNK, 128).T
    yc[:, 64:128] = Y[:, 1].reshape(NCHUNK, 128).T

    x1t = X[:, 1].reshape(NCHUNK, 128).T
    in_maps = []
    for m in range(NCORES):
        bcm = np.empty((128, 80), np.float32)
        bcm[:, 0:64] = x1t
        bcm[:, 64:80] = ys[m * JS:(m + 1) * JS][None, :]
        in_maps.append({"bc": bcm, "ac": ac, "yc": yc})
    return in_maps


def run_on_cores(X, Y, **spmd_kwargs):
    """Run the SPMD kernel; returns BassKernelResults."""
    from concourse.bass_utils import run_bass_kernel_spmd

    nc = _get_program()
    in_maps = _host_inputs(X, Y)
    res = run_bass_kernel_spmd(nc, in_maps, core_ids=list(range(NCORES)),
                               **spmd_kwargs)
    return res


def kernel(X, Y):
    res = run_on_cores(X, Y)
    full = np.empty((3, N_AXIS, N_AXIS), dtype=np.float32)
    for m, r in enumerate(res.results):
        blk = r["out"]  # [128, 48] rows = i, cols = (c, j_local)
        for c in range(3):
            full[c, m * JS:(m + 1) * JS, :] = blk[:, c * JS:(c + 1) * JS].T
    return full



# revision 3
# speedup vs baseline: 1.3633x; 1.3633x over previous
"""ConvCNP encoder kernel for 8x TRN2 NeuronCores.

Math: the reference computes, for a 128x128 uniform grid g=(xs[i], ys[j]) and
n=8192 data points X (2-D) with values psi(Y) = [1, Y0, Y1]:

    Gram[g, x] = exp(-0.5*||g - X[x]||^2)
    fm = Gram @ psi                  # (G, 3); column 0 == row-sum (denominator)
    out[c, j, i] = fm[(i, j), c], with c=1,2 normalized by column 0.

The squared distance is separable over the grid axes:

    Gram[(i,j), x] = A[i, x] * B[j, x]
      A[i, x] = exp(-0.5*(xs[i] - X0[x])^2)     B[j, x] = exp(-0.5*(ys[j] - X1[x])^2)

so, with Bc = B * psi_c (row-wise):  fm[(i,j), c] = sum_x Bc[j, x] * A[i, x].

Sharding: the CONTRACTION axis (the 8192 points) is split across the 8 cores
- 1024 points (8 chunks of 128) per core; grid replicated. Each core computes
its partial fm over its point set:

    acc[i, (c,j)] = sum over 8 x-chunks of  AT_k^T @ BfT_k      (PE, PSUM accum)
      AT_k  = exp(-0.5*(xs[i] - X0[x])^2)   in SBUF layout [x_part=128, i=128]
      BfT_k = [B | B*Y0 | B*Y1]             in SBUF layout [x_part=128, 384]

and the host sums the 8 partial [128, 384] blocks (the unshard step for
contraction sharding), then normalizes c=1,2 by c=0. This cuts the per-core
elementwise (sqdiff + exp) volume 4.5x vs grid-sharding - from 1.18M to 262K
elements - which is what dominated the grid-sharded kernel's span.

AT/BfT are produced by a fused custom DVE op  sq(Src0 - Src1)  over broadcast
APs (one 1x pass) followed by ACT Exp (scale=-0.5) per stripe, stored as fp16
(fp32 argument keeps exp accuracy; fp16 feeds the PE at 1 col/cycle).
"""

import os

os.environ.setdefault("TRNINF_ENABLE_CUSTOMCOMMS_RDH_AG", "0")

import numpy as np
from contextlib import ExitStack

N_AXIS = 128          # grid points per axis
NPTS = 8192           # data points
NCORES = 8
CPTS = NPTS // NCORES  # 1024 points per core
NCHUNK = CPTS // 128   # 8 contraction chunks of 128
GRID_LO, GRID_HI = -2.0, 2.0
SEM_CAP = 176          # walrus --max-sem-num (restore-tail length)

_CACHE = {}


def _register_sqdiff():
    """Register a fused (a-b)^2 custom DVE op (idempotent)."""
    from concourse import dve_ops
    from concourse.dve_spec import Spec, Src0, Src1, sq, lower
    from concourse.dve_uop import DveOpSpec

    name = "TENSOR_SQDIFF_X"
    for op in dve_ops.OPS:
        if op.name == name:
            return op
    spec = Spec(
        body=sq(Src0 - Src1),
        reference=lambda in0, in1, s0, s1, imm2: (in0.astype(np.float32) - in1) ** 2,
    )
    opcode = max(dve_ops._SUB_OPCODE_FOR_NAME.values()) + 1
    assert opcode < 0x20
    dve_ops._SUB_OPCODE_FOR_NAME[name] = opcode
    shas = {}
    for ver in ("v3", "v4"):
        s = DveOpSpec(name=name, opcode=opcode, uops=lower(spec, ver=ver), rd1_en=True)
        shas[ver] = s.sha(ver)
    op = dve_ops.DveOp(name, spec, subdim=False, uops_sha=shas)
    dve_ops.OPS.append(op)
    dve_ops.CUSTOM_DVE_SPECS[name] = spec
    return op


def _patch_walrus_flags():
    """Cap the compiler's semaphore file so the NEFF epilogue restores fewer
    semaphores (the per-sem restore is pure tail latency inside the measured
    window). Idempotent."""
    import concourse.bass_utils as bu

    if getattr(bu.run_command, "_sem_cap_patched", False):
        return
    orig = bu.run_command

    def run_command_capped(argv, **kwargs):
        if argv and "walrus_driver" in str(argv[0]) and any(
                str(a).startswith("--neff-output-filename") for a in argv):
            argv = list(argv) + [f"--max-sem-num={SEM_CAP}"]
        return orig(argv, **kwargs)

    run_command_capped._sem_cap_patched = True
    bu.run_command = run_command_capped


def _build_program():
    import concourse.bacc as bacc
    import concourse.mybir as mybir
    import concourse.tile as tile

    _patch_walrus_flags()
    sqdiff = _register_sqdiff()

    f32 = mybir.dt.float32
    f16 = mybir.dt.float16
    nc = bacc.Bacc("TRN2", target_bir_lowering=False, debug=False, num_devices=NCORES,
                   enable_partition_id=False, monotonic_sem_count=0)

    # Packed inputs (fewer, earlier DMAs):
    #   ac [128, 272] f32: xsb(0:128) | ysb(128:256) | x0t(256:264) | x1t(264:272)
    #   yc [128, 16]  f16: y0t(0:8) | y1t(8:16)
    ac = nc.dram_tensor("ac", [128, 272], f32, kind="ExternalInput")
    yc = nc.dram_tensor("yc", [128, 16], f16, kind="ExternalInput")
    out = nc.dram_tensor("out", [128, 384], f32, kind="ExternalOutput")

    with tile.TileContext(nc) as tc, ExitStack() as ctx:
        singles = ctx.enter_context(tc.tile_pool(name="singles", bufs=1))
        psum = ctx.enter_context(tc.tile_pool(name="psum", bufs=1, space="PSUM"))

        s_ac = singles.tile([128, 272], f32, tag="ac")
        nc.sync.dma_start(s_ac[:, :], ac[:, :])
        s_yc = singles.tile([128, 16], f16, tag="yc")
        nc.gpsimd.dma_start(s_yc[:, :], yc[:, :])

        xsb = s_ac[:, 0:128]
        ysb = s_ac[:, 128:256]
        x0t = s_ac[:, 256:264]
        x1t = s_ac[:, 264:272]

        s_argB = singles.tile([128, NCHUNK, 128], f32, tag="argB")
        s_argA = singles.tile([128, NCHUNK, 128], f32, tag="argA")
        s_bf = singles.tile([128, NCHUNK, 384], f16, tag="bf")
        s_at = singles.tile([128, NCHUNK, 128], f16, tag="at")
        acc = psum.tile([128, 384], f32, tag="acc")

        stripes = [(0, 4), (4, 4)]

        # Emission follows dataflow order (the tile framework derives deps
        # from emission order); per-engine execution order is:
        #   DVE: argB s0,s1 | argA s0,s1 | mul s0,s1
        #   ACT: expB s0,s1 | expA s0,s1
        for (k0, w) in stripes:
            nc.vector._custom_dve(
                sqdiff,
                out=s_argB[:, k0:k0 + w, :],
                in0=ysb.unsqueeze(1).broadcast_to([128, w, 128]),
                in1=x1t[:, k0:k0 + w].unsqueeze(2).broadcast_to([128, w, 128]),
            )
        for (k0, w) in stripes:
            nc.scalar.activation(
                s_bf[:, k0:k0 + w, 0:128], s_argB[:, k0:k0 + w, :],
                mybir.ActivationFunctionType.Exp, scale=-0.5,
            )
        for (k0, w) in stripes:
            nc.vector._custom_dve(
                sqdiff,
                out=s_argA[:, k0:k0 + w, :],
                in0=xsb.unsqueeze(1).broadcast_to([128, w, 128]),
                in1=x0t[:, k0:k0 + w].unsqueeze(2).broadcast_to([128, w, 128]),
            )
        # yck[x, k, c] view of yc[x, c*8+k]
        yck = s_yc[:, :].rearrange("p (c k) -> p k c", c=2)
        for (k0, w) in stripes:
            nc.vector.tensor_tensor(
                s_bf[:, k0:k0 + w, 128:384].rearrange("p w (c j) -> p w c j", c=2),
                s_bf[:, k0:k0 + w, 0:128].unsqueeze(2)
                    .broadcast_to([128, w, 2, 128]),
                yck[:, k0:k0 + w, :].unsqueeze(3).broadcast_to([128, w, 2, 128]),
                mybir.AluOpType.mult,
            )
        for (k0, w) in stripes:
            nc.scalar.activation(
                s_at[:, k0:k0 + w, :], s_argA[:, k0:k0 + w, :],
                mybir.ActivationFunctionType.Exp, scale=-0.5,
            )

        # ---- PE: 8 accumulating matmuls acc[i, (c,j)] ----
        for k in range(NCHUNK):
            nc.tensor.matmul(
                acc[:, :],
                s_at[:, k, :],   # stationary lhsT: [128x, 128i] fp16
                s_bf[:, k, :],   # moving rhs: [128x, 384] fp16
                start=(k == 0),
                stop=(k == NCHUNK - 1),
            )

        # ---- epilogue: PSUM -> SBUF -> HBM (partial sums; host reduces) ----
        s_out = singles.tile([128, 384], f32, tag="outt")
        nc.vector.tensor_copy(s_out[:, :], acc[:, :])
        nc.sync.dma_start(out[:, :], s_out[:, :])

    nc.finalize()
    return nc


def _get_program():
    if "nc" not in _CACHE:
        _CACHE["nc"] = _build_program()
    return _CACHE["nc"]


def _host_inputs(X, Y):
    """Build the per-core input maps (layout prep only)."""
    X = np.ascontiguousarray(np.asarray(X, dtype=np.float32))
    Y = np.ascontiguousarray(np.asarray(Y, dtype=np.float32))
    xs = np.linspace(GRID_LO, GRID_HI, N_AXIS, dtype=np.float32)
    ys = np.linspace(GRID_LO, GRID_HI, N_AXIS, dtype=np.float32)

    in_maps = []
    for m in range(NCORES):
        sl = slice(m * CPTS, (m + 1) * CPTS)
        acm = np.empty((128, 272), np.float32)
        acm[:, 0:128] = xs[None, :]
        acm[:, 128:256] = ys[None, :]
        acm[:, 256:264] = X[sl, 0].reshape(NCHUNK, 128).T
        acm[:, 264:272] = X[sl, 1].reshape(NCHUNK, 128).T
        ycm = np.empty((128, 16), np.float16)
        ycm[:, 0:8] = Y[sl, 0].reshape(NCHUNK, 128).T
        ycm[:, 8:16] = Y[sl, 1].reshape(NCHUNK, 128).T
        in_maps.append({"ac": acm, "yc": ycm})
    return in_maps


def run_on_cores(X, Y, **spmd_kwargs):
    """Run the SPMD kernel; returns BassKernelResults."""
    from concourse.bass_utils import run_bass_kernel_spmd

    nc = _get_program()
    in_maps = _host_inputs(X, Y)
    res = run_bass_kernel_spmd(nc, in_maps, core_ids=list(range(NCORES)),
                               **spmd_kwargs)
    return res


def kernel(X, Y):
    res = run_on_cores(X, Y)
    # Sum the per-core partial contractions (contraction-axis unshard).
    fm = np.zeros((128, 384), dtype=np.float32)
    for r in res.results:
        fm += r["out"]
    full = np.empty((3, N_AXIS, N_AXIS), dtype=np.float32)
    den = fm[:, 0:128]
    full[0] = den.T
    full[1] = (fm[:, 128:256] / den).T
    full[2] = (fm[:, 256:384] / den).T
    return full


# revision 4
# speedup vs baseline: 1.4236x; 1.0442x over previous
"""ConvCNP encoder kernel for 8x TRN2 NeuronCores.

Math: the reference computes, for a 128x128 uniform grid g=(xs[i], ys[j]) and
n=8192 data points X (2-D) with values psi(Y) = [1, Y0, Y1]:

    Gram[g, x] = exp(-0.5*||g - X[x]||^2)
    fm = Gram @ psi                  # (G, 3); column 0 == row-sum (denominator)
    out[c, j, i] = fm[(i, j), c], with c=1,2 normalized by column 0.

The squared distance is separable over the grid axes:

    Gram[(i,j), x] = A[i, x] * B[j, x]
      A[i, x] = exp(-0.5*(xs[i] - X0[x])^2)     B[j, x] = exp(-0.5*(ys[j] - X1[x])^2)

so, with Bc = B * psi_c (row-wise):  fm[(i,j), c] = sum_x Bc[j, x] * A[i, x].

Sharding: the CONTRACTION axis (the 8192 points) is split across the 8 cores
- 1024 points (8 chunks of 128) per core; grid replicated. Each core computes
its partial fm over its point set:

    acc[i, (c,j)] = sum over 8 x-chunks of  AT_k^T @ BfT_k      (PE, PSUM accum)
      AT_k  = exp(-0.5*(xs[i] - X0[x])^2)   in SBUF layout [x_part=128, i=128]
      BfT_k = [B | B*Y0 | B*Y1]             in SBUF layout [x_part=128, 384]

and the host sums the 8 partial [128, 384] blocks (the unshard step for
contraction sharding), then normalizes c=1,2 by c=0.

Engine split per core (balancing DVE / ACT / PE):
  - AT's exponent comes from the PE as a K=3 outer product
        T[x,i] = X0[x]*xs[i] - 0.5*X0[x]^2 - 0.5*xs[i]^2   (PSUM f32)
    followed by a striped ACT Exp -> fp16.  This keeps the big [x, i] sqdiff
    off the DVE entirely.
  - BfT's B part is a fused custom DVE op sq(Src0 - Src1) + ACT Exp.
  - The B*Yc muls run on the DVE at the 2x packed 16-bit rate: Y is shipped
    8x-replicated ([x, k, c, jl] with jl=8 packed) so ALL mul operands have a
    stride-1 16-bit last dim - a stride-0 broadcast in the last dim would
    drop the DVE to the 1x fallback path.
"""

import numpy as np
from contextlib import ExitStack

N_AXIS = 128          # grid points per axis
NPTS = 8192           # data points
NCORES = 8
CPTS = NPTS // NCORES  # 1024 points per core
NCHUNK = CPTS // 128   # 8 contraction chunks of 128
GRID_LO, GRID_HI = -2.0, 2.0

_CACHE = {}


def _register_sqdiff():
    """Register a fused (a-b)^2 custom DVE op (idempotent)."""
    from concourse import dve_ops
    from concourse.dve_spec import Spec, Src0, Src1, sq, lower
    from concourse.dve_uop import DveOpSpec

    name = "TENSOR_SQDIFF_X"
    for op in dve_ops.OPS:
        if op.name == name:
            return op
    spec = Spec(
        body=sq(Src0 - Src1),
        reference=lambda in0, in1, s0, s1, imm2: (in0.astype(np.float32) - in1) ** 2,
    )
    opcode = max(dve_ops._SUB_OPCODE_FOR_NAME.values()) + 1
    assert opcode < 0x20
    dve_ops._SUB_OPCODE_FOR_NAME[name] = opcode
    shas = {}
    for ver in ("v3", "v4"):
        s = DveOpSpec(name=name, opcode=opcode, uops=lower(spec, ver=ver), rd1_en=True)
        shas[ver] = s.sha(ver)
    op = dve_ops.DveOp(name, spec, subdim=False, uops_sha=shas)
    dve_ops.OPS.append(op)
    dve_ops.CUSTOM_DVE_SPECS[name] = spec
    return op


def _build_program():
    import concourse.bacc as bacc
    import concourse.mybir as mybir
    import concourse.tile as tile

    sqdiff = _register_sqdiff()

    f32 = mybir.dt.float32
    f16 = mybir.dt.float16
    nc = bacc.Bacc("TRN2", target_bir_lowering=False, debug=False, num_devices=NCORES,
                   enable_partition_id=False, monotonic_sem_count=0)

    # Packed inputs (one DMA per queue, all issued immediately):
    #   ac [128, 136] f32: ysb(0:128) | x1t(128:136)           (sync queue)
    #   y8 [128, 128] f16: Y 8x-replicated, col = k*16+c*8+jl  (pool queue)
    #   pm [4, 1152]  f16: outer-product operands              (act queue)
    #        rows 0:3, cols k*128+x : [X0 | -0.5*X0^2 | 1] chunk k (stationary)
    #        rows 0:3, cols 1024:1152: [xs; 1; -0.5*xs^2]         (moving)
    ac = nc.dram_tensor("ac", [128, 136], f32, kind="ExternalInput")
    y8 = nc.dram_tensor("y8", [128, 128], f16, kind="ExternalInput")
    pm = nc.dram_tensor("pm", [4, 1152], f16, kind="ExternalInput")
    out = nc.dram_tensor("out", [128, 384], f32, kind="ExternalOutput")

    with tile.TileContext(nc) as tc, ExitStack() as ctx:
        singles = ctx.enter_context(tc.tile_pool(name="singles", bufs=1))
        psum = ctx.enter_context(tc.tile_pool(name="psum", bufs=1, space="PSUM"))

        s_ac = singles.tile([128, 136], f32, tag="ac")
        nc.sync.dma_start(s_ac[:, :], ac[:, :])
        s_y8 = singles.tile([128, 128], f16, tag="y8")
        nc.gpsimd.dma_start(s_y8[:, :], y8[:, :])
        s_pm = singles.tile([4, 1152], f16, tag="pm")
        nc.scalar.dma_start(s_pm[:, :], pm[:, :])

        ysb = s_ac[:, 0:128]
        x1t = s_ac[:, 128:136]

        s_argB = singles.tile([128, NCHUNK, 128], f32, tag="argB")
        s_bf = singles.tile([128, NCHUNK, 384], f16, tag="bf")
        s_at = singles.tile([128, NCHUNK, 128], f16, tag="at")
        psA = psum.tile([128, NCHUNK, 128], f32, tag="psA")
        acc = psum.tile([128, 384], f32, tag="acc")

        stripes = [(0, 4), (4, 4)]

        # ---- PE: A-exponent outer products (chunk k -> PSUM) ----
        for k in range(NCHUNK):
            nc.tensor.matmul(
                psA[:, k, :],
                s_pm[0:3, k * 128:(k + 1) * 128],   # lhsT [3, 128x]
                s_pm[0:3, 1024:1152],               # rhs  [3, 128i]
                start=True, stop=True,
            )

        # ---- DVE argB / ACT exps / DVE muls, striped ----
        for (k0, w) in stripes:
            nc.vector._custom_dve(
                sqdiff,
                out=s_argB[:, k0:k0 + w, :],
                in0=ysb.unsqueeze(1).broadcast_to([128, w, 128]),
                in1=x1t[:, k0:k0 + w].unsqueeze(2).broadcast_to([128, w, 128]),
            )

        # y8 view [x, k, c, jl]
        y8v = s_y8[:, :].rearrange("p (k c jl) -> p k c jl", c=2, jl=8)

        def emit_expB(k0, w):
            nc.scalar.activation(
                s_bf[:, k0:k0 + w, 0:128], s_argB[:, k0:k0 + w, :],
                mybir.ActivationFunctionType.Exp, scale=-0.5,
            )

        def emit_expA(k0, w):
            nc.scalar.activation(
                s_at[:, k0:k0 + w, :], psA[:, k0:k0 + w, :],
                mybir.ActivationFunctionType.Exp,
            )

        def emit_muls(k0, w):
            # bf[:, k, 128+c*128+j] = B[x,k,j] * Y_c[x,k]; j = jh*8+jl.
            # All operands keep a packed 16-bit last dim (jl) for DVE 2x.
            for c in range(2):
                nc.vector.tensor_tensor(
                    s_bf[:, k0:k0 + w, 128 + c * 128:256 + c * 128]
                        .rearrange("p w (jh jl) -> p w jh jl", jl=8),
                    s_bf[:, k0:k0 + w, 0:128]
                        .rearrange("p w (jh jl) -> p w jh jl", jl=8),
                    y8v[:, k0:k0 + w, c, :].unsqueeze(2)
                        .broadcast_to([128, w, 16, 8]),
                    mybir.AluOpType.mult,
                )

        emit_expB(*stripes[0])
        emit_expA(*stripes[0])
        emit_muls(*stripes[0])
        emit_expB(*stripes[1])
        emit_muls(*stripes[1])
        emit_expA(*stripes[1])

        # ---- PE: 8 accumulating matmuls acc[i, (c,j)] ----
        for k in range(NCHUNK):
            nc.tensor.matmul(
                acc[:, :],
                s_at[:, k, :],   # stationary lhsT: [128x, 128i] fp16
                s_bf[:, k, :],   # moving rhs: [128x, 384] fp16
                start=(k == 0),
                stop=(k == NCHUNK - 1),
            )

        # ---- epilogue: PSUM -> SBUF -> HBM (partial sums; host reduces) ----
        s_out = singles.tile([128, 384], f32, tag="outt")
        nc.vector.tensor_copy(s_out[:, :], acc[:, :])
        nc.sync.dma_start(out[:, :], s_out[:, :])

    nc.finalize()
    return nc


def _get_program():
    if "nc" not in _CACHE:
        _CACHE["nc"] = _build_program()
    return _CACHE["nc"]


def _host_inputs(X, Y):
    """Build the per-core input maps (layout prep only)."""
    X = np.ascontiguousarray(np.asarray(X, dtype=np.float32))
    Y = np.ascontiguousarray(np.asarray(Y, dtype=np.float32))
    xs = np.linspace(GRID_LO, GRID_HI, N_AXIS, dtype=np.float32)
    ys = np.linspace(GRID_LO, GRID_HI, N_AXIS, dtype=np.float32)

    in_maps = []
    for m in range(NCORES):
        sl = slice(m * CPTS, (m + 1) * CPTS)
        x0 = X[sl, 0]
        acm = np.empty((128, 136), np.float32)
        acm[:, 0:128] = ys[None, :]
        acm[:, 128:136] = X[sl, 1].reshape(NCHUNK, 128).T
        # y8[x, k*16 + c*8 + jl] = Y[chunk k, point x, c]
        y8m = np.empty((128, NCHUNK, 2, 8), np.float16)
        y8m[:, :, 0, :] = Y[sl, 0].reshape(NCHUNK, 128).T[:, :, None]
        y8m[:, :, 1, :] = Y[sl, 1].reshape(NCHUNK, 128).T[:, :, None]
        pmm = np.zeros((4, 1152), np.float16)
        pmm[0, 0:1024] = x0
        pmm[1, 0:1024] = -0.5 * x0.astype(np.float64) ** 2
        pmm[2, 0:1024] = 1.0
        pmm[0, 1024:1152] = xs
        pmm[1, 1024:1152] = 1.0
        pmm[2, 1024:1152] = -0.5 * xs.astype(np.float64) ** 2
        in_maps.append({"ac": acm, "y8": y8m.reshape(128, 128), "pm": pmm})
    return in_maps


def run_on_cores(X, Y, **spmd_kwargs):
    """Run the SPMD kernel; returns BassKernelResults."""
    from concourse.bass_utils import run_bass_kernel_spmd

    nc = _get_program()
    in_maps = _host_inputs(X, Y)
    res = run_bass_kernel_spmd(nc, in_maps, core_ids=list(range(NCORES)),
                               **spmd_kwargs)
    return res


def kernel(X, Y):
    res = run_on_cores(X, Y)
    # Sum the per-core partial contractions (contraction-axis unshard).
    fm = np.zeros((128, 384), dtype=np.float32)
    for r in res.results:
        fm += r["out"]
    full = np.empty((3, N_AXIS, N_AXIS), dtype=np.float32)
    den = fm[:, 0:128]
    full[0] = den.T
    full[1] = (fm[:, 128:256] / den).T
    full[2] = (fm[:, 256:384] / den).T
    return full
